# revision 35
# baseline (speedup 1.0000x reference)
"""Trainium2 Bass kernel for the EntangledInterferenceLayer problem (v2).

Math transformations done on host (numpy), all exact up to fp rounding:
  * The HxH entanglement mix commutes with RoPE (cos/sin are head-independent),
    so it folds into the Q/K projection weights + biases.
  * The per-head phase shift rotates q and k by the same complex phase, and the
    attention logits use q * conj(k) -> the phase cancels exactly.  Dropped.
  * 1/sqrt(head_dim) folds into the Q weights/bias.
  * The V-projection bias contributes bv @ Wo to every output row (softmax rows
    sum to 1), so it folds into the output bias.

Sharding (8 cores): core = (batch b, head-group g of 4 heads). Each core
projects Q/K/V for its heads, runs causal complex-magnitude attention, then
AllGathers the normalized attention outputs within its 4-core batch group
(split into vr/vi halves per 512-token chunk so the output projections can
start after a quarter of the gathered bytes) and computes a 256-column slice
of both output projections.

Perf notes (vs v1, 490us -> ~305us):
  * bf16 operands everywhere: LDWEIGHTS runs at FWL rate (149us -> 62us),
    DMA bytes and the gathered payload halve.
  * sqrt/exp each issued as ONE activation instruction per q-chunk over a
    contiguous [128, h, kv, 512] magnitude tile - the tile scheduler then
    cannot interleave sqrt/exp and thrash the ACT table (~29 x 1.3us).
  * psi staging copies split across ACT (qc0) / DVE (qc1) to balance the
    attention-phase engine load; QK/V staging DMAs issued from the GPSIMD
    queue so the Sync queue isn't the projection-phase bottleneck.
  * x / lt loads coalesced into single rearranged DMAs.
  * QK (non-rot) and output-projection biases folded into the PSUM->SBUF
    Identity-activation copies (per-partition bias), dropping 16 matmuls.
  * softmax denominators: batched on 32-aligned partition rows, one DVE
    reciprocal per chunk, row-select matmul broadcast (fp32), no DRAM
    round trip.  (reciprocal_approx_fast and gpsimd partition_broadcast
    both return garbage on this hardware despite correct sim results.)
"""

import math

import numpy as np

B, S, DIM = 2, 1024, 1024
HEADS, HD, ROTD = 16, 64, 32
GH = 4  # heads per core
NCORES = 8

_PAIRSWAP = [i ^ 1 for i in range(32)]

# 'bfloat16' (fast weight loads) or 'float32r' (exact-ish, full-rate matmul)
MM_DTYPE = "bfloat16"


def _register_magsq():
    """Register a fused custom DVE op: out = (in0^2 + in1^2) * imm2."""
    import numpy as np
    from concourse import dve_ops as DO
    from concourse.dve_spec import Spec, Src0, Src1, C2, sq, lower

    if "ANT_MAGSQ" in DO._SUB_OPCODE_FOR_NAME:
        return next(o for o in DO.OPS if o.name == "ANT_MAGSQ")
    spec = Spec(
        body=(sq(Src0) + sq(Src1)) * C2,
        reference=lambda in0, in1, s0, s1, imm2: (
            in0.astype(np.float32) ** 2 + in1.astype(np.float32) ** 2
        )
        * np.float32(imm2),
    )
    opcode = DO._CUSTOM_DVE_ROW_BASE + len(DO.OPS)
    DO._SUB_OPCODE_FOR_NAME["ANT_MAGSQ"] = opcode
    shas = {}
    for ver in ("v3", "v4"):
        try:
            s = DO.DveOpSpec(
                name="ANT_MAGSQ", opcode=opcode, uops=lower(spec, ver=ver), rd1_en=True
            )
            shas[ver] = s.sha(ver)
        except Exception:
            pass
    op = DO.DveOp("ANT_MAGSQ", spec, subdim=False, uops_sha=shas)
    DO.OPS.append(op)
    DO.CUSTOM_DVE_SPECS["ANT_MAGSQ"] = spec
    return op


DEBUG_DUMP = False


def _build(gt: float, groups=None, mm_dtype=None):
    import concourse.mybir as mybir
    import concourse.tile as tile
    from concourse import bacc

    f32 = mybir.dt.float32
    f32r = mybir.dt.float32r
    mdt = getattr(mybir.dt, mm_dtype or MM_DTYPE)
    AF = mybir.ActivationFunctionType
    magsq = _register_magsq()

    nc = bacc.Bacc("TRN2", target_bir_lowering=False, num_devices=NCORES)
    if groups is None:
        groups = [[0, 1, 2, 3], [4, 5, 6, 7]]

    xr = nc.dram_tensor("xr_t", [DIM, S], mdt, kind="ExternalInput")
    xi = nc.dram_tensor("xi_t", [DIM, S], mdt, kind="ExternalInput")
    w = {
        nm: nc.dram_tensor(nm, [DIM, 256], mdt, kind="ExternalInput")
        for nm in ["wqr", "wqi", "wkr", "wki", "wvr", "wvi"]
    }
    wo_d = {
        nm: nc.dram_tensor(nm, [DIM, 256], mdt, kind="ExternalInput")
        for nm in ["wor", "woi"]
    }
    bias_d = {
        nm: nc.dram_tensor(nm, [1, 256], mdt, kind="ExternalInput")
        for nm in ["bqr", "bqi", "bkr", "bki", "bor", "boi"]
    }
    bcol_d = {
        nm: nc.dram_tensor("c" + nm, [128, 2], f32, kind="ExternalInput")
        for nm in ["bqr", "bqi", "bkr", "bki", "bor", "boi"]
    }
    cosd = nc.dram_tensor("cosd", [128, S], f32, kind="ExternalInput")
    seld = nc.dram_tensor("seld", [128, 256], f32, kind="ExternalInput")
    sind = nc.dram_tensor("sind", [128, S], f32, kind="ExternalInput")
    # out.T layout like v1: [out-dim slice, token]
    o_r = nc.dram_tensor("o_r", [256, S], f32, kind="ExternalOutput")
    o_i = nc.dram_tensor("o_i", [256, S], f32, kind="ExternalOutput")
    if DEBUG_DUMP:
        dbg_den = nc.dram_tensor("dbg_den", [8, 512], f32, kind="ExternalOutput")
        dbg_rec = nc.dram_tensor("dbg_rec", [8, 512], f32, kind="ExternalOutput")
        dbg_agin = nc.dram_tensor("dbg_agin", [256, 512], mdt, kind="ExternalOutput")
        dbg_agout = nc.dram_tensor(
            "dbg_agout", [4, 256, 512], mdt, kind="ExternalOutput"
        )

    def mm(out, lhsT, rhs, start, stop):
        nc.tensor.matmul(out, lhsT=lhsT, rhs=rhs, start=start, stop=stop)

    with tile.TileContext(nc) as tc:
        with (
            tc.tile_pool(name="consts", bufs=1) as consts,
            tc.tile_pool(name="xp", bufs=3) as xp,
            tc.tile_pool(name="wqk", bufs=8) as wqkp,
            tc.tile_pool(name="wv", bufs=1) as wvp,
            tc.tile_pool(name="wo", bufs=1) as wop,
            tc.tile_pool(name="persist", bufs=1) as persist,
            tc.tile_pool(name="mag0", bufs=1) as mag0p,
            tc.tile_pool(name="mag1", bufs=1) as mag1p,
            tc.tile_pool(name="stage", bufs=2) as stage,
            tc.tile_pool(name="evp", bufs=2) as evp,
            tc.tile_pool(name="small", bufs=2) as small,
            tc.tile_pool(name="lop", bufs=4) as lop,
            tc.tile_pool(name="ps", bufs=8, space="PSUM") as ps,
            tc.tile_pool(name="dram", bufs=1, space="DRAM") as dram,
        ):
            # ---- x input loads (c=0 first: shortest path to first matmul;
            # c=1 tiles are emitted inside the c loop so the pool can recycle
            # the c=0 buffers once the c=0 projections have consumed them) ----
            x_t = {"r": [None, None], "i": [None, None]}

            def load_x(c):
                csl = slice(c * 512, (c + 1) * 512)
                for key, src in (("r", xr), ("i", xi)):
                    t = xp.tile([128, 8, 512], mdt, tag="x", name=f"x{key}{c}")
                    nc.sync.dma_start(
                        t, src[:, csl].rearrange("(kt kp) c -> kp kt c", kp=128)
                    )
                    x_t[key][c] = t

            # ---- QK weights: all 8 (proj, mt) tiles, loaded once.  The
            # first projection's weight tile goes before the big x DMAs so
            # the first matmul can start as early as possible. ----
            wqk_sb = {}

            def load_wqk(wname, mt, eng=None):
                t = wqkp.tile([128, 8, 128], mdt, tag="wqk", name=f"{wname}{mt}")
                (eng or nc.sync).dma_start(
                    t,
                    w[wname][:, mt * 128 : (mt + 1) * 128].rearrange(
                        "(kt kp) m -> kp kt m", kp=128
                    ),
                )
                wqk_sb[(wname, mt)] = t

            load_wqk("wqr", 0)
            load_wqk("wqr", 1, nc.scalar)
            load_x(0)
            # alternate queues so the strided weight gathers issue in parallel
            for i, (wname, mt) in enumerate(
                (w_, m_) for w_ in ["wqr", "wqi", "wkr", "wki"] for m_ in range(2)
            ):
                if (wname, mt) not in wqk_sb:
                    load_wqk(wname, mt, nc.scalar if i % 2 else nc.sync)
            # V weights fused [wvr | wvi] along the moving free dim.
            wv_sb = wvp.tile([128, 8, 512], mdt, tag="wv")
            nc.sync.dma_start(
                wv_sb[:, :, 0:256], w["wvr"].rearrange("(kt kp) m -> kp kt m", kp=128)
            )
            nc.sync.dma_start(
                wv_sb[:, :, 256:512],
                w["wvi"].rearrange("(kt kp) m -> kp kt m", kp=128),
            )

            # ---- constants ----
            ones_t = consts.tile([1, 512], mdt, tag="ones")
            nc.vector.memset(ones_t, 1.0)
            eps_t = consts.tile([128, 1], f32, tag="eps")
            nc.vector.memset(eps_t, 1e-6 * float(gt) * float(gt))
            cos_sb = consts.tile([128, S], f32, tag="cos")
            nc.sync.dma_start(cos_sb, cosd[:, :])
            sin_sb = consts.tile([128, S], f32, tag="sin")
            nc.sync.dma_start(sin_sb, sind[:, :])
            sel_sb = consts.tile([128, 256], f32, tag="sel")
            nc.sync.dma_start(sel_sb, seld[:, :])
            bcol_sb = {}
            for nm in bcol_d:
                t = consts.tile([128, 2], f32, tag="c" + nm, name=f"c{nm}")
                nc.sync.dma_start(t, bcol_d[nm][:, :])
                bcol_sb[nm] = t
            bias_sb = {}
            for nm in bias_d:
                t = consts.tile([1, 256], mdt, tag=nm, name=f"b{nm}")
                nc.sync.dma_start(t, bias_d[nm][:, :])
                bias_sb[nm] = t
            Q = persist.tile([128, GH, S], mdt, tag="Q")
            K1 = persist.tile([128, GH, S], mdt, tag="K1")
            K2 = persist.tile([128, GH, S], mdt, tag="K2")
            Vr = persist.tile([128, 8, GH, 65], mdt, tag="Vr")
            Vi = persist.tile([128, 8, GH, 64], mdt, tag="Vi")
            nc.vector.memset(Vr[:, :, :, 64:65], 1.0)

            # one magnitude/weights tile per q-chunk: [128, h, kvt, 512].
            # A single sqrt (resp. exp) instruction covers the whole chunk, so
            # the scheduler cannot interleave sqrt/exp and thrash ACT tables.
            mag = {
                0: mag0p.tile([128, GH, 4, 512], mdt, tag="m0", name="m0"),
                1: mag1p.tile([128, GH, 8, 512], mdt, tag="m1", name="m1"),
            }

            # (name, x-source key, weight, bias, rot-targets, nonrot-targets)
            projs = [
                ("qr", "r", "wqr", "bqr", [(0, 0)], [(0, 32)]),
                ("qi", "i", "wqi", "bqi", [(0, 64)], [(0, 96)]),
                ("kr", "r", "wkr", "bkr", [(1, 0), (2, 64)], [(1, 32), (2, 96)]),
                ("ki", "i", "wki", "bki", [(1, 64)], [(1, 96)]),
            ]
            qk_tensors = {0: Q, 1: K1, 2: K2}
            gt2 = float(gt) * float(gt)

            for c in range(2):
                if c == 1:
                    load_x(1)
                csl = slice(c * 512, (c + 1) * 512)
                for pname, xkey, wname, bname, rot_tgts, nr_tgts in projs:
                    for mt in range(2):  # 0 = rot dims, 1 = non-rot dims
                        w_sb = wqk_sb[(wname, mt)]
                        pst = ps.tile([128, 512], f32, tag="ps")
                        for kt in range(8):
                            mm(pst, w_sb[:, kt, :], x_t[xkey][c][:, kt, :],
                               start=(kt == 0), stop=(mt == 1 and kt == 7))
                        if mt == 0:
                            # rope mixes tokens, so the bias must land before
                            # the rotation: add it with a K=1 matmul.
                            mm(pst, bias_sb[bname][:, 0:128],
                               ones_t, start=False, stop=True)
                        if mt == 0:
                            shuf = stage.tile([128, 512], f32, tag="shuf")
                            nc.vector.stream_shuffle(shuf, pst, mask=_PAIRSWAP)
                            nc.vector.tensor_mul(shuf, shuf, sin_sb[:, csl])
                            t2 = stage.tile([128, 512], mdt, tag="t2")
                            nc.vector.tensor_mul(t2, pst, cos_sb[:, csl])
                            nc.vector.tensor_add(t2, t2, shuf)
                            src_t = t2
                        else:
                            evn = evp.tile([128, 512], mdt, tag="ev")
                            nc.scalar.activation(evn, pst, AF.Identity,
                                                 bias=bcol_sb[bname][:, 1:2])
                            src_t = evn
                        tgts = rot_tgts if mt == 0 else nr_tgts
                        for tid, row0 in tgts:
                            dst = qk_tensors[tid]
                            # K staging rides the (otherwise idle) GPSIMD
                            # queue so the Sync queue doesn't serialize the
                            # whole projection phase on DMA issue.
                            eng = nc.sync if tid == 0 else nc.gpsimd
                            for h in range(GH):
                                eng.dma_start(
                                    dst[row0 : row0 + 32, h, csl],
                                    src_t[h * 32 : (h + 1) * 32, :],
                                )
                        if pname == "ki":  # negated copy into K2 rows 0:32/32:64
                            neg = evp.tile([128, 512], mdt, tag="ev")
                            nc.vector.tensor_scalar_mul(neg, src_t, -1.0)
                            row0 = 0 if mt == 0 else 32
                            for h in range(GH):
                                nc.gpsimd.dma_start(
                                    K2[row0 : row0 + 32, h, csl],
                                    neg[h * 32 : (h + 1) * 32, :],
                                )

                # V projections: x tile stationary, wv half as moving operand.
                for xkey, w0, Vt in (("r", 0, Vr), ("i", 256, Vi)):
                    for tl in range(4):
                        tt = c * 4 + tl
                        pv = ps.tile([128, 256], f32, tag="ps")
                        for kt in range(8):
                            mm(pv,
                               x_t[xkey][c][:, kt, tl * 128 : (tl + 1) * 128],
                               wv_sb[:, kt, w0 : w0 + 256],
                               start=(kt == 0), stop=(kt == 7))
                        ov = evp.tile([128, 256], mdt, tag="ov")
                        nc.scalar.copy(ov, pv)
                        nc.gpsimd.dma_start(
                            Vt[:, tt, :, 0:64],
                            ov.rearrange("p (h d) -> p h d", h=GH),
                        )


                # scores + |z|^2 for q-chunk qc=c (kv tiles 0..(c+1)*4-1)
                qc = c
                qcs = slice(qc * 512, (qc + 1) * 512)
                nkv = (qc + 1) * 4
                for h in range(GH):
                    qsl = Q[:, h, qcs]
                    for kvt in range(nkv):
                        ksl = slice(kvt * 128, (kvt + 1) * 128)
                        psr = ps.tile([128, 512], f32, tag="ps")
                        mm(psr, K1[:, h, ksl], qsl, start=True, stop=True)
                        psi = ps.tile([128, 512], f32, tag="ps")
                        mm(psi, K2[:, h, ksl], qsl, start=True, stop=True)
                        # HW allows only one PSUM operand per instruction:
                        # stage psi through SBUF, then fuse square-add-scale.
                        c1 = stage.tile([128, 512], f32, tag="c1")
                        # engine split keeps both ACT and DVE below the
                        # attention-phase critical path (GPSIMD can't see PSUM)
                        if qc == 0:
                            nc.scalar.copy(c1, psi)
                        else:
                            nc.vector.tensor_copy(c1, psi)
                        nc.vector._custom_dve(
                            magsq,
                            out=mag[qc][:, h, kvt, :],
                            in0=psr, in1=c1, imm2=gt2,
                        )

            # ---- output-projection weights (large; loads overlap attention) --
            wo_sb = {}
            for ri, wname in ((0, "wor"), (1, "woi")):
                t = wop.tile([128, 8, 256], mdt, tag=wname, name=f"wo{ri}")
                nc.sync.dma_start(
                    t, wo_d[wname].rearrange("(kt kp) m -> kp kt m", kp=128)
                )
                wo_sb[ri] = t

            # ---- batched ACT phases: sqrt then exp (few table loads) ----
            def act_batch(qc, func, h0=0, h1=GH, **kw):
                sl = mag[qc][:, h0:h1, :, :]
                nc.scalar.activation(sl, sl, func, **kw)

            def masks(qc, heads):
                nkv = (qc + 1) * 4
                for h in heads:
                    for kvt in range(nkv):
                        off = kvt - qc * 4
                        if off >= 0:
                            sl = mag[qc][:, h, kvt, :]
                            nc.gpsimd.affine_select(
                                out=sl, in_=sl,
                                compare_op=mybir.AluOpType.is_ge,
                                fill=0.0, base=-(off * 128),
                                channel_multiplier=-1, pattern=[[1, 512]],
                            )

            def av_head(qc, h):
                nkv = (qc + 1) * 4
                avr = ps.tile([65, 512], f32, tag="ps", name=f"avr{qc}{h}")
                avi = ps.tile([64, 512], f32, tag="ps", name=f"avi{qc}{h}")
                for kvt in range(nkv):
                    et = mag[qc][:, h, kvt, :]
                    mm(avr, Vr[:, kvt, h, :], et,
                       start=(kvt == 0), stop=(kvt == nkv - 1))
                    mm(avi, Vi[:, kvt, h, :], et,
                       start=(kvt == 0), stop=(kvt == nkv - 1))
                onr = evp.tile([65, 512], f32, tag="on", bufs=4, name=f"onr{qc}{h}")
                nc.scalar.copy(onr, avr)
                oni = evp.tile([64, 512], f32, tag="oni", bufs=4, name=f"oni{qc}{h}")
                nc.scalar.copy(oni, avi)
                return onr, oni

            # Two AllGathers per q-chunk (vr half, vi half): the real-part
            # output projection can start after only the vr gather, and the
            # last exposed piece is a quarter of the total gathered bytes.
            agin = [
                dram.tile([512, 512], mdt, tag=f"agin{qc}", name=f"agin{qc}")
                for qc in range(2)
            ]
            agout = {
                (qc, part): dram.tile(
                    [4, 256, 512], mdt, tag=f"agout{qc}{part}",
                    name=f"agout{qc}{part}"
                )
                for qc in range(2)
                for part in ("r", "i")
            }

            def norm_and_gather(qc, ons):
                # softmax denominators batched on 32-aligned partition rows ->
                # one DVE reciprocal -> GPSIMD partition_broadcast per head.
                den = small.tile([128, 512], f32, tag="den", bufs=2,
                                 name=f"den{qc}")
                nc.gpsimd.memset(den, 1.0)
                for h in range(GH):
                    nc.gpsimd.tensor_copy(
                        den[32 * h : 32 * h + 1, :], ons[h][0][64:65, :]
                    )
                rec = small.tile([128, 512], f32, tag="rec", bufs=2,
                                 name=f"rec{qc}")
                nc.vector.reciprocal(rec, den)
                if DEBUG_DUMP:
                    for h in range(GH):
                        nc.sync.dma_start(
                            dbg_rec[qc * 4 + h : qc * 4 + h + 1, :],
                            rec[32 * h : 32 * h + 1, :],
                        )
                        nc.sync.dma_start(
                            dbg_den[qc * 4 + h : qc * 4 + h + 1, :],
                            den[32 * h : 32 * h + 1, :],
                        )
                bcs = []
                for h in range(GH):
                    onr, oni = ons[h]
                    bc = ps.tile([64, 512], f32, tag="ps", name=f"bc{qc}{h}")
                    mm(bc, sel_sb[:, h * 64 : (h + 1) * 64], rec,
                       start=True, stop=True)
                    bcs.append(bc)
                    onn = evp.tile([64, 512], mdt, tag="onn", bufs=4,
                                   name=f"onn{qc}{h}")
                    nc.vector.tensor_mul(onn, onr[0:64, :], bc)
                    nc.gpsimd.dma_start(agin[qc][64 * h : 64 * (h + 1), :], onn)
                nc.gpsimd.collective_compute(
                    "AllGather",
                    mybir.AluOpType.bypass,
                    replica_groups=groups,
                    ins=[agin[qc][0:256, :].opt()],
                    outs=[agout[(qc, "r")][:].opt()],
                )
                for h in range(GH):
                    onr, oni = ons[h]
                    onn2 = evp.tile([64, 512], mdt, tag="onn", bufs=4,
                                    name=f"onn2{qc}{h}")
                    nc.vector.tensor_mul(onn2, oni, bcs[h])
                    nc.gpsimd.dma_start(
                        agin[qc][256 + 64 * h : 256 + 64 * (h + 1), :], onn2
                    )
                nc.gpsimd.collective_compute(
                    "AllGather",
                    mybir.AluOpType.bypass,
                    replica_groups=groups,
                    ins=[agin[qc][256:512, :].opt()],
                    outs=[agout[(qc, "i")][:].opt()],
                )

            act_batch(0, AF.Sqrt, bias=eps_t)
            act_batch(0, AF.Exp, scale=1.0)
            masks(0, range(GH))
            ons0 = [av_head(0, h) for h in range(GH)]
            norm_and_gather(0, ons0)
            act_batch(1, AF.Sqrt, h0=0, h1=2, bias=eps_t)
            act_batch(1, AF.Exp, h0=0, h1=2, scale=1.0)
            masks(1, (0, 1))
            ons1 = [av_head(1, h) for h in (0, 1)]
            act_batch(1, AF.Sqrt, h0=2, h1=4, bias=eps_t)
            act_batch(1, AF.Exp, h0=2, h1=4, scale=1.0)
            masks(1, (2, 3))
            ons1 += [av_head(1, h) for h in (2, 3)]
            norm_and_gather(1, ons1)

            if DEBUG_DUMP:
                nc.sync.dma_start(dbg_agin[:, :], agin[0][0:256, :])
                nc.sync.dma_start(dbg_agout[:, :, :], agout[0][:, 0:256, :])

            # ---- output projection (v1 orientation: out.T[od, tok]) ----
            # kt = p*4 + g indexes the gathered 128-row blocks; Wo rows were
            # permuted on the host to match this order.
            for qc in (0, 1):
                for ri, part, bname, odst in (
                    (0, "r", "bor", o_r),
                    (1, "i", "boi", o_i),
                ):
                    ltg = []
                    for g in range(4):
                        t = lop.tile([128, 2, 512], mdt, tag="lt",
                                     name=f"lt{qc}{part}{g}")
                        nc.sync.dma_start(
                            t,
                            agout[(qc, part)][g, :, :].rearrange(
                                "(p kp) t -> kp p t", kp=128
                            ),
                        )
                        ltg.append(t)
                    lt = [ltg[kt // 2][:, kt % 2, :] for kt in range(8)]
                    pos = [ps.tile([128, 512], f32, tag="ps", name=f"po{qc}{ri}{i}")
                           for i in range(2)]
                    for kt in range(8):
                        for odt in range(2):
                            mm(pos[odt],
                               wo_sb[ri][:, kt, odt * 128 : (odt + 1) * 128],
                               lt[kt], start=(kt == 0), stop=(kt == 7))
                    for odt in range(2):
                        oo = evp.tile([128, 512], f32, tag="oo")
                        nc.scalar.activation(oo, pos[odt], AF.Identity,
                                             bias=bcol_sb[bname][:, odt : odt + 1])
                        nc.sync.dma_start(
                            odst[odt * 128 : (odt + 1) * 128,
                                 qc * 512 : (qc + 1) * 512],
                            oo,
                        )

    return nc


_SELD = np.zeros((128, 256), np.float32)
for _h in range(4):
    _SELD[32 * _h, _h * 64 : (_h + 1) * 64] = 1.0


def _host_prep(inputs):
    """Fold ent/scale/bv on host; build per-core input maps."""
    import ml_dtypes

    mdt_np = (
        ml_dtypes.bfloat16 if MM_DTYPE == "bfloat16" else np.float32
    )
    f = lambda x: np.asarray(x, dtype=np.float32)
    real, imag = f(inputs["real"]), f(inputs["imag"])
    ent = np.asarray(inputs["ent"], np.float64)
    scale = 1.0 / math.sqrt(HD)

    def fold_w(W, do_ent, sc=1.0):
        W = np.asarray(W, np.float64).reshape(DIM, HEADS, HD)
        if do_ent:
            W = np.einsum("chd,hx->cxd", W, ent)
        return W * sc  # [DIM, HEADS, HD] float64

    def fold_b(b, do_ent, sc=1.0):
        b = np.asarray(b, np.float64).reshape(HEADS, HD)
        if do_ent:
            b = np.einsum("hd,hx->xd", b, ent)
        return b * sc

    Wq_r = fold_w(inputs["Wq_r"], True, scale)
    Wq_i = fold_w(inputs["Wq_i"], True, scale)
    Wk_r = fold_w(inputs["Wk_r"], True)
    Wk_i = fold_w(inputs["Wk_i"], True)
    Wv_r = fold_w(inputs["Wv_r"], False)
    Wv_i = fold_w(inputs["Wv_i"], False)
    bq_r = fold_b(inputs["bq_r"], True, scale)
    bq_i = fold_b(inputs["bq_i"], True, scale)
    bk_r = fold_b(inputs["bk_r"], True)
    bk_i = fold_b(inputs["bk_i"], True)
    Wo_r = np.asarray(inputs["Wo_r"], np.float64)
    Wo_i = np.asarray(inputs["Wo_i"], np.float64)
    bo_r = np.asarray(inputs["bo_r"], np.float64) + np.asarray(
        inputs["bv_r"], np.float64
    ) @ Wo_r
    bo_i = np.asarray(inputs["bo_i"], np.float64) + np.asarray(
        inputs["bv_i"], np.float64
    ) @ Wo_i

    strength = float(np.asarray(inputs["strength"]).reshape(-1)[0])
    temp = float(np.asarray(inputs["temp"]).reshape(-1)[0])
    gt = (1.0 / (1.0 + math.exp(-strength))) / max(temp, 0.01)

    # rope tables in device layout: row h*32+d (d<32), freq j=d//2
    rot_freqs = np.asarray(inputs["rot_freqs"], np.float64)  # [16]
    pos = np.arange(S, dtype=np.float64)
    emb = pos[:, None] * rot_freqs[None, :]  # [S, 16]
    cos_t = np.cos(emb)
    sin_t = np.sin(emb)
    cosd = np.empty((128, S), np.float32)
    sind = np.empty((128, S), np.float32)
    for hh in range(4):
        for d in range(32):
            r = hh * 32 + d
            cosd[r] = cos_t[:, d // 2]
            sind[r] = (-sin_t if d % 2 == 0 else sin_t)[:, d // 2]

    def qk_dev(Wf, bf, g):
        # [DIM,H,HD]/[H,HD] -> per-core [DIM,256]/[1,256] in [rot x 4h | nr x 4h]
        hs = slice(g * GH, (g + 1) * GH)
        Wc, bc = Wf[:, hs, :], bf[hs, :]
        wd = np.concatenate(
            [
                Wc[:, :, :ROTD].reshape(DIM, GH * ROTD),
                Wc[:, :, ROTD:].reshape(DIM, GH * ROTD),
            ],
            axis=1,
        )
        bd = np.concatenate(
            [bc[:, :ROTD].reshape(1, GH * ROTD), bc[:, ROTD:].reshape(1, GH * ROTD)],
            axis=1,
        )
        return wd.astype(mdt_np), bd.astype(mdt_np)

    in_maps = []
    for core in range(NCORES):
        b, g = core // 4, core % 4
        hs = slice(g * GH, (g + 1) * GH)
        m = {
            "xr_t": np.ascontiguousarray(real[b].T).astype(mdt_np),
            "xi_t": np.ascontiguousarray(imag[b].T).astype(mdt_np),
            "cosd": cosd,
            "sind": sind,
            "seld": _SELD,
            "wvr": np.ascontiguousarray(Wv_r[:, hs, :].reshape(DIM, 256)).astype(
                mdt_np
            ),
            "wvi": np.ascontiguousarray(Wv_i[:, hs, :].reshape(DIM, 256)).astype(
                mdt_np
            ),
            "wor": np.ascontiguousarray(
                Wo_r[:, g * 256 : (g + 1) * 256]
            ).astype(mdt_np),
            "woi": np.ascontiguousarray(
                Wo_i[:, g * 256 : (g + 1) * 256]
            ).astype(mdt_np),
            "bor": bo_r[None, g * 256 : (g + 1) * 256].astype(mdt_np),
            "boi": bo_i[None, g * 256 : (g + 1) * 256].astype(mdt_np),
        }
        for nm, Wf, bf in (
            ("qr", Wq_r, bq_r),
            ("qi", Wq_i, bq_i),
            ("kr", Wk_r, bk_r),
            ("ki", Wk_i, bk_i),
        ):
            wd, bd = qk_dev(Wf, bf, g)
            m["w" + nm] = wd
            m["b" + nm] = bd
            # column layout for the ACT-bias path (non-rot half at [:, 1])
            m["cb" + nm] = (
                np.asarray(bd, np.float32).reshape(2, 128).T.copy()
            )
        m["cbor"] = np.asarray(
            bo_r[g * 256 : (g + 1) * 256], np.float32
        ).reshape(2, 128).T.copy()
        m["cboi"] = np.asarray(
            bo_i[g * 256 : (g + 1) * 256], np.float32
        ).reshape(2, 128).T.copy()
        in_maps.append(m)
    return in_maps, gt


def _assemble(results):
    out_r = np.empty((B, S, DIM), np.float32)
    out_i = np.empty((B, S, DIM), np.float32)
    for core in range(NCORES):
        b, g = core // 4, core % 4
        out_r[b, :, g * 256 : (g + 1) * 256] = results[core]["o_r"].T
        out_i[b, :, g * 256 : (g + 1) * 256] = results[core]["o_i"].T
    return np.stack([out_r, out_i], axis=0)


def kernel(**inputs):
    from concourse import bass_utils

    in_maps, gt = _host_prep(inputs)
    nc = _build(gt)
    nc.finalize()
    res = bass_utils.run_bass_kernel_spmd(
        nc, in_maps, core_ids=list(range(NCORES))
    )
    return _assemble(res.results)


# revision 36
# speedup vs baseline: 1.1079x; 1.1079x over previous
"""Trainium2 Bass kernel for the EntangledInterferenceLayer problem (v2).

Math transformations done on host (numpy), all exact up to fp rounding:
  * The HxH entanglement mix commutes with RoPE (cos/sin are head-independent),
    so it folds into the Q/K projection weights + biases.
  * The per-head phase shift rotates q and k by the same complex phase, and the
    attention logits use q * conj(k) -> the phase cancels exactly.  Dropped.
  * 1/sqrt(head_dim) folds into the Q weights/bias.
  * The V-projection bias contributes bv @ Wo to every output row (softmax rows
    sum to 1), so it folds into the output bias.

Sharding (8 cores): core = (batch b, head-group g of 4 heads). Each core
projects Q/K/V for its heads, runs causal complex-magnitude attention, then
AllGathers the normalized attention outputs within its 4-core batch group
(split into vr/vi halves per 512-token chunk so the output projections can
start after a quarter of the gathered bytes) and computes a 256-column slice
of both output projections.

Perf notes (vs v1, 490us -> ~305us):
  * bf16 operands everywhere: LDWEIGHTS runs at FWL rate (149us -> 62us),
    DMA bytes and the gathered payload halve.
  * sqrt/exp each issued as ONE activation instruction per q-chunk over a
    contiguous [128, h, kv, 512] magnitude tile - the tile scheduler then
    cannot interleave sqrt/exp and thrash the ACT table (~29 x 1.3us).
  * psi staging copies split across ACT (qc0) / DVE (qc1) to balance the
    attention-phase engine load; QK/V staging DMAs issued from the GPSIMD
    queue so the Sync queue isn't the projection-phase bottleneck.
  * x / lt loads coalesced into single rearranged DMAs.
  * QK (non-rot) and output-projection biases folded into the PSUM->SBUF
    Identity-activation copies (per-partition bias), dropping 16 matmuls.
  * softmax denominators: batched on 32-aligned partition rows, one DVE
    reciprocal per chunk, row-select matmul broadcast (fp32), no DRAM
    round trip.  (reciprocal_approx_fast and gpsimd partition_broadcast
    both return garbage on this hardware despite correct sim results.)
"""

import math

import numpy as np

B, S, DIM = 2, 1024, 1024
HEADS, HD, ROTD = 16, 64, 32
GH = 4  # heads per core
NCORES = 8

_PAIRSWAP = [i ^ 1 for i in range(32)]

# 'bfloat16' (fast weight loads) or 'float32r' (exact-ish, full-rate matmul)
MM_DTYPE = "bfloat16"


def _register_magsq():
    """Register a fused custom DVE op: out = (in0^2 + in1^2) * imm2."""
    import numpy as np
    from concourse import dve_ops as DO
    from concourse.dve_spec import Spec, Src0, Src1, C2, sq, lower

    if "ANT_MAGSQ" in DO._SUB_OPCODE_FOR_NAME:
        return next(o for o in DO.OPS if o.name == "ANT_MAGSQ")
    spec = Spec(
        body=(sq(Src0) + sq(Src1)) * C2,
        reference=lambda in0, in1, s0, s1, imm2: (
            in0.astype(np.float32) ** 2 + in1.astype(np.float32) ** 2
        )
        * np.float32(imm2),
    )
    opcode = DO._CUSTOM_DVE_ROW_BASE + len(DO.OPS)
    DO._SUB_OPCODE_FOR_NAME["ANT_MAGSQ"] = opcode
    shas = {}
    for ver in ("v3", "v4"):
        try:
            s = DO.DveOpSpec(
                name="ANT_MAGSQ", opcode=opcode, uops=lower(spec, ver=ver), rd1_en=True
            )
            shas[ver] = s.sha(ver)
        except Exception:
            pass
    op = DO.DveOp("ANT_MAGSQ", spec, subdim=False, uops_sha=shas)
    DO.OPS.append(op)
    DO.CUSTOM_DVE_SPECS["ANT_MAGSQ"] = spec
    return op


DEBUG_DUMP = False


def _build(gt: float, groups=None, mm_dtype=None):
    import concourse.mybir as mybir
    import concourse.tile as tile
    from concourse import bacc

    f32 = mybir.dt.float32
    f32r = mybir.dt.float32r
    mdt = getattr(mybir.dt, mm_dtype or MM_DTYPE)
    AF = mybir.ActivationFunctionType
    magsq = _register_magsq()

    nc = bacc.Bacc("TRN2", target_bir_lowering=False, num_devices=NCORES)
    if groups is None:
        groups = [[0, 1, 2, 3], [4, 5, 6, 7]]

    xr = nc.dram_tensor("xr_t", [DIM, S], mdt, kind="ExternalInput")
    xi = nc.dram_tensor("xi_t", [DIM, S], mdt, kind="ExternalInput")
    w = {
        nm: nc.dram_tensor(nm, [DIM, 256], mdt, kind="ExternalInput")
        for nm in ["wqr", "wqi", "wkr", "wki", "wvr", "wvi"]
    }
    wo_d = {
        nm: nc.dram_tensor(nm, [DIM, 256], mdt, kind="ExternalInput")
        for nm in ["wor", "woi"]
    }
    bias_d = {
        nm: nc.dram_tensor(nm, [1, 256], mdt, kind="ExternalInput")
        for nm in ["bqr", "bqi", "bkr", "bki", "bor", "boi"]
    }
    bcol_d = {
        nm: nc.dram_tensor("c" + nm, [128, 2], f32, kind="ExternalInput")
        for nm in ["bqr", "bqi", "bkr", "bki", "bor", "boi"]
    }
    cosd = nc.dram_tensor("cosd", [128, S], f32, kind="ExternalInput")
    seld = nc.dram_tensor("seld", [128, 256], f32, kind="ExternalInput")
    sind = nc.dram_tensor("sind", [128, S], f32, kind="ExternalInput")
    # out.T layout like v1: [out-dim slice, token]
    o_r = nc.dram_tensor("o_r", [256, S], f32, kind="ExternalOutput")
    o_i = nc.dram_tensor("o_i", [256, S], f32, kind="ExternalOutput")
    if DEBUG_DUMP:
        dbg_den = nc.dram_tensor("dbg_den", [8, 512], f32, kind="ExternalOutput")
        dbg_rec = nc.dram_tensor("dbg_rec", [8, 512], f32, kind="ExternalOutput")
        dbg_agin = nc.dram_tensor("dbg_agin", [256, 512], mdt, kind="ExternalOutput")
        dbg_agout = nc.dram_tensor(
            "dbg_agout", [4, 256, 512], mdt, kind="ExternalOutput"
        )

    def mm(out, lhsT, rhs, start, stop):
        nc.tensor.matmul(out, lhsT=lhsT, rhs=rhs, start=start, stop=stop)

    with tile.TileContext(nc) as tc:
        with (
            tc.tile_pool(name="consts", bufs=1) as consts,
            tc.tile_pool(name="xp", bufs=2) as xp,
            tc.tile_pool(name="wqk", bufs=8) as wqkp,
            tc.tile_pool(name="wv", bufs=1) as wvp,
            tc.tile_pool(name="wo", bufs=1) as wop,
            tc.tile_pool(name="persist", bufs=1) as persist,
            tc.tile_pool(name="mag0", bufs=1) as mag0p,
            tc.tile_pool(name="mag1", bufs=1) as mag1p,
            tc.tile_pool(name="stage", bufs=2) as stage,
            tc.tile_pool(name="evp", bufs=2) as evp,
            tc.tile_pool(name="small", bufs=2) as small,
            tc.tile_pool(name="lop", bufs=4) as lop,
            tc.tile_pool(name="ps", bufs=8, space="PSUM") as ps,
            tc.tile_pool(name="dram", bufs=1, space="DRAM") as dram,
        ):
            # ---- x input loads (c=0 first: shortest path to first matmul;
            # c=1 tiles are emitted inside the c loop so the pool can recycle
            # the c=0 buffers once the c=0 projections have consumed them) ----
            x_t = {"r": [None, None], "i": [None, None]}

            def load_x(c):
                csl = slice(c * 512, (c + 1) * 512)
                for key, src in (("r", xr), ("i", xi)):
                    t = xp.tile([128, 8, 512], mdt, tag="x", name=f"x{key}{c}")
                    nc.sync.dma_start(
                        t, src[:, csl].rearrange("(kt kp) c -> kp kt c", kp=128)
                    )
                    x_t[key][c] = t

            # ---- QK weights: all 8 (proj, mt) tiles, loaded once.  The
            # first projection's weight tile goes before the big x DMAs so
            # the first matmul can start as early as possible. ----
            wqk_sb = {}

            def load_wqk(wname, mt, eng=None):
                t = wqkp.tile([128, 8, 128], mdt, tag="wqk", name=f"{wname}{mt}")
                (eng or nc.sync).dma_start(
                    t,
                    w[wname][:, mt * 128 : (mt + 1) * 128].rearrange(
                        "(kt kp) m -> kp kt m", kp=128
                    ),
                )
                wqk_sb[(wname, mt)] = t

            load_wqk("wqr", 0)
            load_x(0)
            for wname in ["wqr", "wqi", "wkr", "wki"]:
                for mt in range(2):
                    if (wname, mt) not in wqk_sb:
                        load_wqk(wname, mt)
            # V weights fused [wvr | wvi] along the moving free dim.
            wv_sb = wvp.tile([128, 8, 512], mdt, tag="wv")
            nc.sync.dma_start(
                wv_sb[:, :, 0:256], w["wvr"].rearrange("(kt kp) m -> kp kt m", kp=128)
            )
            nc.sync.dma_start(
                wv_sb[:, :, 256:512],
                w["wvi"].rearrange("(kt kp) m -> kp kt m", kp=128),
            )

            # ---- constants ----
            ones_t = consts.tile([1, 512], mdt, tag="ones")
            nc.vector.memset(ones_t, 1.0)
            eps_t = consts.tile([128, 1], f32, tag="eps")
            nc.vector.memset(eps_t, 1e-6 * float(gt) * float(gt))
            cos_sb = consts.tile([128, S], f32, tag="cos")
            nc.sync.dma_start(cos_sb, cosd[:, :])
            sin_sb = consts.tile([128, S], f32, tag="sin")
            nc.sync.dma_start(sin_sb, sind[:, :])
            sel_sb = consts.tile([128, 256], f32, tag="sel")
            nc.sync.dma_start(sel_sb, seld[:, :])
            bcol_sb = {}
            for nm in bcol_d:
                t = consts.tile([128, 2], f32, tag="c" + nm, name=f"c{nm}")
                nc.sync.dma_start(t, bcol_d[nm][:, :])
                bcol_sb[nm] = t
            bias_sb = {}
            for nm in bias_d:
                t = consts.tile([1, 256], mdt, tag=nm, name=f"b{nm}")
                nc.sync.dma_start(t, bias_d[nm][:, :])
                bias_sb[nm] = t
            Q = persist.tile([128, GH, S], mdt, tag="Q")
            K1 = persist.tile([128, GH, S], mdt, tag="K1")
            K2 = persist.tile([128, GH, S], mdt, tag="K2")
            Vr = persist.tile([128, 8, GH, 65], mdt, tag="Vr")
            Vi = persist.tile([128, 8, GH, 64], mdt, tag="Vi")
            nc.vector.memset(Vr[:, :, :, 64:65], 1.0)

            # one magnitude/weights tile per q-chunk: [128, h, kvt, 512].
            # A single sqrt (resp. exp) instruction covers the whole chunk, so
            # the scheduler cannot interleave sqrt/exp and thrash ACT tables.
            mag = {
                0: mag0p.tile([128, GH, 4, 512], mdt, tag="m0", name="m0"),
                1: mag1p.tile([128, GH, 8, 512], mdt, tag="m1", name="m1"),
            }

            # (name, x-source key, weight, bias, rot-targets, nonrot-targets)
            projs = [
                ("qr", "r", "wqr", "bqr", [(0, 0)], [(0, 32)]),
                ("qi", "i", "wqi", "bqi", [(0, 64)], [(0, 96)]),
                ("kr", "r", "wkr", "bkr", [(1, 0), (2, 64)], [(1, 32), (2, 96)]),
                ("ki", "i", "wki", "bki", [(1, 64)], [(1, 96)]),
            ]
            qk_tensors = {0: Q, 1: K1, 2: K2}
            gt2 = float(gt) * float(gt)

            for c in range(2):
                if c == 1:
                    load_x(1)
                csl = slice(c * 512, (c + 1) * 512)
                for pname, xkey, wname, bname, rot_tgts, nr_tgts in projs:
                    for mt in range(2):  # 0 = rot dims, 1 = non-rot dims
                        w_sb = wqk_sb[(wname, mt)]
                        pst = ps.tile([128, 512], f32, tag="ps")
                        for kt in range(8):
                            mm(pst, w_sb[:, kt, :], x_t[xkey][c][:, kt, :],
                               start=(kt == 0), stop=(mt == 1 and kt == 7))
                        if mt == 0:
                            # rope mixes tokens, so the bias must land before
                            # the rotation: add it with a K=1 matmul.
                            mm(pst, bias_sb[bname][:, 0:128],
                               ones_t, start=False, stop=True)
                        if mt == 0:
                            shuf = stage.tile([128, 512], f32, tag="shuf")
                            nc.vector.stream_shuffle(shuf, pst, mask=_PAIRSWAP)
                            nc.vector.tensor_mul(shuf, shuf, sin_sb[:, csl])
                            t2 = stage.tile([128, 512], mdt, tag="t2")
                            nc.vector.tensor_mul(t2, pst, cos_sb[:, csl])
                            nc.vector.tensor_add(t2, t2, shuf)
                            src_t = t2
                        else:
                            evn = evp.tile([128, 512], mdt, tag="ev")
                            nc.scalar.activation(evn, pst, AF.Identity,
                                                 bias=bcol_sb[bname][:, 1:2])
                            src_t = evn
                        tgts = rot_tgts if mt == 0 else nr_tgts
                        for tid, row0 in tgts:
                            dst = qk_tensors[tid]
                            # K staging rides the (otherwise idle) GPSIMD
                            # queue so the Sync queue doesn't serialize the
                            # whole projection phase on DMA issue.
                            eng = nc.sync if tid == 0 else nc.gpsimd
                            for h in range(GH):
                                eng.dma_start(
                                    dst[row0 : row0 + 32, h, csl],
                                    src_t[h * 32 : (h + 1) * 32, :],
                                )
                        if pname == "ki":  # negated copy into K2 rows 0:32/32:64
                            neg = evp.tile([128, 512], mdt, tag="ev")
                            nc.vector.tensor_scalar_mul(neg, src_t, -1.0)
                            row0 = 0 if mt == 0 else 32
                            for h in range(GH):
                                nc.gpsimd.dma_start(
                                    K2[row0 : row0 + 32, h, csl],
                                    neg[h * 32 : (h + 1) * 32, :],
                                )

                # V projections: x tile stationary, wv half as moving operand.
                for xkey, w0, Vt in (("r", 0, Vr), ("i", 256, Vi)):
                    for tl in range(4):
                        tt = c * 4 + tl
                        pv = ps.tile([128, 256], f32, tag="ps")
                        for kt in range(8):
                            mm(pv,
                               x_t[xkey][c][:, kt, tl * 128 : (tl + 1) * 128],
                               wv_sb[:, kt, w0 : w0 + 256],
                               start=(kt == 0), stop=(kt == 7))
                        ov = evp.tile([128, 256], mdt, tag="ov")
                        nc.scalar.copy(ov, pv)
                        nc.gpsimd.dma_start(
                            Vt[:, tt, :, 0:64],
                            ov.rearrange("p (h d) -> p h d", h=GH),
                        )


                # scores + |z|^2 for q-chunk qc=c (kv tiles 0..(c+1)*4-1)
                qc = c
                qcs = slice(qc * 512, (qc + 1) * 512)
                nkv = (qc + 1) * 4
                for h in range(GH):
                    qsl = Q[:, h, qcs]
                    for kvt in range(nkv):
                        ksl = slice(kvt * 128, (kvt + 1) * 128)
                        psr = ps.tile([128, 512], f32, tag="ps")
                        mm(psr, K1[:, h, ksl], qsl, start=True, stop=True)
                        psi = ps.tile([128, 512], f32, tag="ps")
                        mm(psi, K2[:, h, ksl], qsl, start=True, stop=True)
                        # HW allows only one PSUM operand per instruction:
                        # stage psi through SBUF, then fuse square-add-scale.
                        c1 = stage.tile([128, 512], f32, tag="c1")
                        # engine split keeps both ACT and DVE below the
                        # attention-phase critical path (GPSIMD can't see PSUM)
                        if qc == 0:
                            nc.scalar.copy(c1, psi)
                        else:
                            nc.vector.tensor_copy(c1, psi)
                        nc.vector._custom_dve(
                            magsq,
                            out=mag[qc][:, h, kvt, :],
                            in0=psr, in1=c1, imm2=gt2,
                        )

            # ---- output-projection weights (large; loads overlap attention) --
            wo_sb = {}
            for ri, wname in ((0, "wor"), (1, "woi")):
                t = wop.tile([128, 8, 256], mdt, tag=wname, name=f"wo{ri}")
                nc.sync.dma_start(
                    t, wo_d[wname].rearrange("(kt kp) m -> kp kt m", kp=128)
                )
                wo_sb[ri] = t

            # ---- batched ACT phases: sqrt then exp (few table loads) ----
            def act_batch(qc, func, h0=0, h1=GH, **kw):
                sl = mag[qc][:, h0:h1, :, :]
                nc.scalar.activation(sl, sl, func, **kw)

            def masks(qc, heads):
                nkv = (qc + 1) * 4
                for h in heads:
                    for kvt in range(nkv):
                        off = kvt - qc * 4
                        if off >= 0:
                            sl = mag[qc][:, h, kvt, :]
                            nc.gpsimd.affine_select(
                                out=sl, in_=sl,
                                compare_op=mybir.AluOpType.is_ge,
                                fill=0.0, base=-(off * 128),
                                channel_multiplier=-1, pattern=[[1, 512]],
                            )

            def av_head(qc, h):
                nkv = (qc + 1) * 4
                avr = ps.tile([65, 512], f32, tag="ps", name=f"avr{qc}{h}")
                avi = ps.tile([64, 512], f32, tag="ps", name=f"avi{qc}{h}")
                for kvt in range(nkv):
                    et = mag[qc][:, h, kvt, :]
                    mm(avr, Vr[:, kvt, h, :], et,
                       start=(kvt == 0), stop=(kvt == nkv - 1))
                    mm(avi, Vi[:, kvt, h, :], et,
                       start=(kvt == 0), stop=(kvt == nkv - 1))
                onr = evp.tile([65, 512], f32, tag="on", bufs=4, name=f"onr{qc}{h}")
                nc.scalar.copy(onr, avr)
                oni = evp.tile([64, 512], f32, tag="oni", bufs=4, name=f"oni{qc}{h}")
                nc.scalar.copy(oni, avi)
                return onr, oni

            # Two AllGathers per q-chunk (vr half, vi half): the real-part
            # output projection can start after only the vr gather, and the
            # last exposed piece is a quarter of the total gathered bytes.
            agin = [
                dram.tile([512, 512], mdt, tag=f"agin{qc}", name=f"agin{qc}")
                for qc in range(2)
            ]
            agout = {
                (qc, part): dram.tile(
                    [4, 256, 512], mdt, tag=f"agout{qc}{part}",
                    name=f"agout{qc}{part}"
                )
                for qc in range(2)
                for part in ("r", "i")
            }

            def norm_and_gather(qc, ons):
                # softmax denominators batched on 32-aligned partition rows ->
                # one DVE reciprocal -> GPSIMD partition_broadcast per head.
                den = small.tile([128, 512], f32, tag="den", bufs=2,
                                 name=f"den{qc}")
                nc.gpsimd.memset(den, 1.0)
                for h in range(GH):
                    nc.gpsimd.tensor_copy(
                        den[32 * h : 32 * h + 1, :], ons[h][0][64:65, :]
                    )
                rec = small.tile([128, 512], f32, tag="rec", bufs=2,
                                 name=f"rec{qc}")
                nc.vector.reciprocal(rec, den)
                if DEBUG_DUMP:
                    for h in range(GH):
                        nc.sync.dma_start(
                            dbg_rec[qc * 4 + h : qc * 4 + h + 1, :],
                            rec[32 * h : 32 * h + 1, :],
                        )
                        nc.sync.dma_start(
                            dbg_den[qc * 4 + h : qc * 4 + h + 1, :],
                            den[32 * h : 32 * h + 1, :],
                        )
                bcs = []
                for h in range(GH):
                    onr, oni = ons[h]
                    bc = ps.tile([64, 512], f32, tag="ps", name=f"bc{qc}{h}")
                    mm(bc, sel_sb[:, h * 64 : (h + 1) * 64], rec,
                       start=True, stop=True)
                    bcs.append(bc)
                    onn = evp.tile([64, 512], mdt, tag="onn", bufs=4,
                                   name=f"onn{qc}{h}")
                    nc.vector.tensor_mul(onn, onr[0:64, :], bc)
                    nc.gpsimd.dma_start(agin[qc][64 * h : 64 * (h + 1), :], onn)
                nc.gpsimd.collective_compute(
                    "AllGather",
                    mybir.AluOpType.bypass,
                    replica_groups=groups,
                    ins=[agin[qc][0:256, :].opt()],
                    outs=[agout[(qc, "r")][:].opt()],
                )
                for h in range(GH):
                    onr, oni = ons[h]
                    onn2 = evp.tile([64, 512], mdt, tag="onn", bufs=4,
                                    name=f"onn2{qc}{h}")
                    nc.vector.tensor_mul(onn2, oni, bcs[h])
                    nc.gpsimd.dma_start(
                        agin[qc][256 + 64 * h : 256 + 64 * (h + 1), :], onn2
                    )
                nc.gpsimd.collective_compute(
                    "AllGather",
                    mybir.AluOpType.bypass,
                    replica_groups=groups,
                    ins=[agin[qc][256:512, :].opt()],
                    outs=[agout[(qc, "i")][:].opt()],
                )

            act_batch(0, AF.Sqrt, bias=eps_t)
            act_batch(0, AF.Exp, scale=1.0)
            masks(0, range(GH))
            ons0 = [av_head(0, h) for h in range(GH)]
            norm_and_gather(0, ons0)
            act_batch(1, AF.Sqrt, h0=0, h1=2, bias=eps_t)
            act_batch(1, AF.Exp, h0=0, h1=2, scale=1.0)
            masks(1, (0, 1))
            ons1 = [av_head(1, h) for h in (0, 1)]
            act_batch(1, AF.Sqrt, h0=2, h1=4, bias=eps_t)
            act_batch(1, AF.Exp, h0=2, h1=4, scale=1.0)
            masks(1, (2, 3))
            ons1 += [av_head(1, h) for h in (2, 3)]
            norm_and_gather(1, ons1)

            if DEBUG_DUMP:
                nc.sync.dma_start(dbg_agin[:, :], agin[0][0:256, :])
                nc.sync.dma_start(dbg_agout[:, :, :], agout[0][:, 0:256, :])

            # ---- output projection (v1 orientation: out.T[od, tok]) ----
            # kt = p*4 + g indexes the gathered 128-row blocks; Wo rows were
            # permuted on the host to match this order.
            for qc in (0, 1):
                for ri, part, bname, odst in (
                    (0, "r", "bor", o_r),
                    (1, "i", "boi", o_i),
                ):
                    ltg = []
                    for g in range(4):
                        t = lop.tile([128, 2, 512], mdt, tag="lt",
                                     name=f"lt{qc}{part}{g}")
                        nc.sync.dma_start(
                            t,
                            agout[(qc, part)][g, :, :].rearrange(
                                "(p kp) t -> kp p t", kp=128
                            ),
                        )
                        ltg.append(t)
                    lt = [ltg[kt // 2][:, kt % 2, :] for kt in range(8)]
                    pos = [ps.tile([128, 512], f32, tag="ps", name=f"po{qc}{ri}{i}")
                           for i in range(2)]
                    for kt in range(8):
                        for odt in range(2):
                            mm(pos[odt],
                               wo_sb[ri][:, kt, odt * 128 : (odt + 1) * 128],
                               lt[kt], start=(kt == 0), stop=(kt == 7))
                    for odt in range(2):
                        oo = evp.tile([128, 512], f32, tag="oo")
                        nc.scalar.activation(oo, pos[odt], AF.Identity,
                                             bias=bcol_sb[bname][:, odt : odt + 1])
                        nc.sync.dma_start(
                            odst[odt * 128 : (odt + 1) * 128,
                                 qc * 512 : (qc + 1) * 512],
                            oo,
                        )

    return nc


_SELD = np.zeros((128, 256), np.float32)
for _h in range(4):
    _SELD[32 * _h, _h * 64 : (_h + 1) * 64] = 1.0


def _host_prep(inputs):
    """Fold ent/scale/bv on host; build per-core input maps."""
    import ml_dtypes

    mdt_np = (
        ml_dtypes.bfloat16 if MM_DTYPE == "bfloat16" else np.float32
    )
    f = lambda x: np.asarray(x, dtype=np.float32)
    real, imag = f(inputs["real"]), f(inputs["imag"])
    ent = np.asarray(inputs["ent"], np.float64)
    scale = 1.0 / math.sqrt(HD)

    def fold_w(W, do_ent, sc=1.0):
        W = np.asarray(W, np.float64).reshape(DIM, HEADS, HD)
        if do_ent:
            W = np.einsum("chd,hx->cxd", W, ent)
        return W * sc  # [DIM, HEADS, HD] float64

    def fold_b(b, do_ent, sc=1.0):
        b = np.asarray(b, np.float64).reshape(HEADS, HD)
        if do_ent:
            b = np.einsum("hd,hx->xd", b, ent)
        return b * sc

    Wq_r = fold_w(inputs["Wq_r"], True, scale)
    Wq_i = fold_w(inputs["Wq_i"], True, scale)
    Wk_r = fold_w(inputs["Wk_r"], True)
    Wk_i = fold_w(inputs["Wk_i"], True)
    Wv_r = fold_w(inputs["Wv_r"], False)
    Wv_i = fold_w(inputs["Wv_i"], False)
    bq_r = fold_b(inputs["bq_r"], True, scale)
    bq_i = fold_b(inputs["bq_i"], True, scale)
    bk_r = fold_b(inputs["bk_r"], True)
    bk_i = fold_b(inputs["bk_i"], True)
    Wo_r = np.asarray(inputs["Wo_r"], np.float64)
    Wo_i = np.asarray(inputs["Wo_i"], np.float64)
    bo_r = np.asarray(inputs["bo_r"], np.float64) + np.asarray(
        inputs["bv_r"], np.float64
    ) @ Wo_r
    bo_i = np.asarray(inputs["bo_i"], np.float64) + np.asarray(
        inputs["bv_i"], np.float64
    ) @ Wo_i

    strength = float(np.asarray(inputs["strength"]).reshape(-1)[0])
    temp = float(np.asarray(inputs["temp"]).reshape(-1)[0])
    gt = (1.0 / (1.0 + math.exp(-strength))) / max(temp, 0.01)

    # rope tables in device layout: row h*32+d (d<32), freq j=d//2
    rot_freqs = np.asarray(inputs["rot_freqs"], np.float64)  # [16]
    pos = np.arange(S, dtype=np.float64)
    emb = pos[:, None] * rot_freqs[None, :]  # [S, 16]
    cos_t = np.cos(emb)
    sin_t = np.sin(emb)
    cosd = np.empty((128, S), np.float32)
    sind = np.empty((128, S), np.float32)
    for hh in range(4):
        for d in range(32):
            r = hh * 32 + d
            cosd[r] = cos_t[:, d // 2]
            sind[r] = (-sin_t if d % 2 == 0 else sin_t)[:, d // 2]

    def qk_dev(Wf, bf, g):
        # [DIM,H,HD]/[H,HD] -> per-core [DIM,256]/[1,256] in [rot x 4h | nr x 4h]
        hs = slice(g * GH, (g + 1) * GH)
        Wc, bc = Wf[:, hs, :], bf[hs, :]
        wd = np.concatenate(
            [
                Wc[:, :, :ROTD].reshape(DIM, GH * ROTD),
                Wc[:, :, ROTD:].reshape(DIM, GH * ROTD),
            ],
            axis=1,
        )
        bd = np.concatenate(
            [bc[:, :ROTD].reshape(1, GH * ROTD), bc[:, ROTD:].reshape(1, GH * ROTD)],
            axis=1,
        )
        return wd.astype(mdt_np), bd.astype(mdt_np)

    in_maps = []
    for core in range(NCORES):
        b, g = core // 4, core % 4
        hs = slice(g * GH, (g + 1) * GH)
        m = {
            "xr_t": np.ascontiguousarray(real[b].T).astype(mdt_np),
            "xi_t": np.ascontiguousarray(imag[b].T).astype(mdt_np),
            "cosd": cosd,
            "sind": sind,
            "seld": _SELD,
            "wvr": np.ascontiguousarray(Wv_r[:, hs, :].reshape(DIM, 256)).astype(
                mdt_np
            ),
            "wvi": np.ascontiguousarray(Wv_i[:, hs, :].reshape(DIM, 256)).astype(
                mdt_np
            ),
            "wor": np.ascontiguousarray(
                Wo_r[:, g * 256 : (g + 1) * 256]
            ).astype(mdt_np),
            "woi": np.ascontiguousarray(
                Wo_i[:, g * 256 : (g + 1) * 256]
            ).astype(mdt_np),
            "bor": bo_r[None, g * 256 : (g + 1) * 256].astype(mdt_np),
            "boi": bo_i[None, g * 256 : (g + 1) * 256].astype(mdt_np),
        }
        for nm, Wf, bf in (
            ("qr", Wq_r, bq_r),
            ("qi", Wq_i, bq_i),
            ("kr", Wk_r, bk_r),
            ("ki", Wk_i, bk_i),
        ):
            wd, bd = qk_dev(Wf, bf, g)
            m["w" + nm] = wd
            m["b" + nm] = bd
            # column layout for the ACT-bias path (non-rot half at [:, 1])
            m["cb" + nm] = (
                np.asarray(bd, np.float32).reshape(2, 128).T.copy()
            )
        m["cbor"] = np.asarray(
            bo_r[g * 256 : (g + 1) * 256], np.float32
        ).reshape(2, 128).T.copy()
        m["cboi"] = np.asarray(
            bo_i[g * 256 : (g + 1) * 256], np.float32
        ).reshape(2, 128).T.copy()
        in_maps.append(m)
    return in_maps, gt


def _assemble(results):
    out_r = np.empty((B, S, DIM), np.float32)
    out_i = np.empty((B, S, DIM), np.float32)
    for core in range(NCORES):
        b, g = core // 4, core % 4
        out_r[b, :, g * 256 : (g + 1) * 256] = results[core]["o_r"].T
        out_i[b, :, g * 256 : (g + 1) * 256] = results[core]["o_i"].T
    return np.stack([out_r, out_i], axis=0)


def kernel(**inputs):
    from concourse import bass_utils

    in_maps, gt = _host_prep(inputs)
    nc = _build(gt)
    nc.finalize()
    res = bass_utils.run_bass_kernel_spmd(
        nc, in_maps, core_ids=list(range(NCORES))
    )
    return _assemble(res.results)
